# revision 28
# baseline (speedup 1.0000x reference)
"""Trainium2 Bass kernel for CombinedKSpaceRowwiseMSELoss.

loss = mean((pred-target)^2 over central cols) summed over both channels'
       means + mean(|pred-target| over periphery cols) likewise.

Strategy: pure data parallel over the batch dim — 32 batches / 8 cores =
4 batches per core. Each core streams its [5120, 640] f32 shard of pred and
target through SBUF in 5 big tiles (128 partitions x 8 rows x 640 cols),
computes diff on the vector engine, fused square+row-accumulate on the vector
engine (tensor_tensor_reduce) for the central columns, and fused
abs+row-accumulate on the scalar engine (activation Abs with accum_out) for
the periphery columns. Per-tile per-partition partial sums land in two small
accumulator tiles that are DMA'd out; the final ~3K-element reduction and
normalization happen on the host.
"""

import sys

for _p in ("/opt/trn_rl_repo",):
    if _p not in sys.path:
        sys.path.insert(0, _p)

import numpy as np
from contextlib import ExitStack

import concourse.bass as bass
import concourse.tile as tile
from concourse import bacc, mybir
from concourse.bass_utils import run_bass_kernel_spmd

N_CORES = 8
B, C, H, W = 32, 2, 640, 640
B_SHARD = B // N_CORES          # 4 batch elements per core
ROWS = B_SHARD * C * H          # 5120 rows per core
P = 128                         # SBUF partitions
RPP = 4                         # rows per partition per big tile
TILE_ROWS = P * RPP             # 1024
T = ROWS // TILE_ROWS           # 5 big tiles per core
CW = int(W * 0.25)              # 160 central cols
CS = (W - CW) // 2              # 240
CE = CS + CW                    # 400
PW = W - CW                     # 480 periphery cols per row

F32 = mybir.dt.float32


def build_program(
    loop_n: int = 1,
    mode: str = "full",
    rpp: int = RPP,
    io_bufs: int = 3,
    rings: str = "split",
) -> bass.Bass:
    T = ROWS // (P * rpp)
    nc = bacc.Bacc("TRN2", target_bir_lowering=False, debug=False)

    pred = nc.dram_tensor("pred", [ROWS, W], F32, kind="ExternalInput")
    tgt = nc.dram_tensor("target", [ROWS, W], F32, kind="ExternalInput")
    cacc_out = nc.dram_tensor("cacc", [P, T], F32, kind="ExternalOutput")
    pacc_out = nc.dram_tensor("pacc", [P, 2 * T], F32, kind="ExternalOutput")

    # Tile t, partition p holds rows t*P*rpp + p*rpp .. +rpp-1, contiguous in DRAM.
    pred_v = pred.ap().rearrange("(t p r) w -> t p (r w)", p=P, r=rpp)
    tgt_v = tgt.ap().rearrange("(t p r) w -> t p (r w)", p=P, r=rpp)

    with tile.TileContext(nc) as tc:
        with ExitStack() as ctx:
            io_pool = ctx.enter_context(tc.tile_pool(name="io", bufs=io_bufs))
            work_pool = ctx.enter_context(tc.tile_pool(name="work", bufs=3))
            acc_pool = ctx.enter_context(tc.tile_pool(name="acc", bufs=1))

            cacc = acc_pool.tile([P, T], F32)
            pacc = acc_pool.tile([P, 2 * T], F32)
            if mode != "full":
                nc.vector.memset(cacc[:], 0.0)
                nc.vector.memset(pacc[:], 0.0)
            fixed_io = None
            if mode not in ("full", "dma"):
                fpred = acc_pool.tile([P, rpp * W], F32, tag="fpred")
                ftgt = acc_pool.tile([P, rpp * W], F32, tag="ftgt")
                fixed_io = (fpred, ftgt)
                nc.vector.memset(fpred[:], 1.0)
                nc.vector.memset(ftgt[:], 2.0)

            def body():
                for t in range(T):
                    emit_tile(t)

            def emit_tile(t):
                # pred on the SP HWDGE ring, target on the ACT HWDGE ring —
                # two issuers so descriptor posting isn't serialized.
                if fixed_io is not None:
                    pt, gt = fixed_io
                else:
                    pt = io_pool.tile([P, rpp * W], F32, tag="pred")
                    gt = io_pool.tile([P, rpp * W], F32, tag="tgt")
                    if rings == "split":
                        nc.sync.dma_start(pt[:], pred_v[t])
                        nc.scalar.dma_start(gt[:], tgt_v[t])
                    elif rings == "sp":
                        nc.sync.dma_start(pt[:], pred_v[t])
                        nc.sync.dma_start(gt[:], tgt_v[t])
                    else:  # alternate rings by tile parity
                        e0, e1 = (nc.sync, nc.scalar) if t % 2 == 0 else (nc.scalar, nc.sync)
                        e0.dma_start(pt[:], pred_v[t])
                        e1.dma_start(gt[:], tgt_v[t])
                if mode == "dma":
                    return

                do_sub = mode in ("full", "compute", "sub", "subsq")
                do_sq = mode in ("full", "compute", "sq", "subsq")
                do_abs = mode in ("full", "compute", "absred")

                if do_sub:
                    diff = work_pool.tile([P, rpp * W], F32, tag="diff")
                    nc.vector.tensor_sub(diff[:], pt[:], gt[:])
                    d3 = diff[:].rearrange("p (r w) -> p r w", w=W)
                else:
                    d3 = gt[:].rearrange("p (r w) -> p r w", w=W)

                if do_sq:
                    # central: accum += sum over rows of (diff*diff) on ACT
                    sq = work_pool.tile([P, rpp * CW], F32, tag="sq")
                    nc.scalar.activation(
                        sq[:].rearrange("p (r w) -> p r w", w=CW),
                        d3[:, :, CS:CE],
                        mybir.ActivationFunctionType.Square,
                        accum_out=cacc[:, t : t + 1],
                    )

                if do_abs:
                    # periphery: abs-sum on DVE (fused |x| reduce, no scratch)
                    nc.vector.tensor_reduce(
                        pacc[:, 2 * t : 2 * t + 1],
                        d3[:, :, 0:CS],
                        axis=mybir.AxisListType.XY,
                        op=mybir.AluOpType.add,
                        apply_absolute_value=True,
                    )
                    nc.vector.tensor_reduce(
                        pacc[:, 2 * t + 1 : 2 * t + 2],
                        d3[:, :, CE:],
                        axis=mybir.AxisListType.XY,
                        op=mybir.AluOpType.add,
                        apply_absolute_value=True,
                    )

            if loop_n > 1:
                with tc.For_i(0, loop_n, 1):
                    body()
            else:
                body()

            nc.sync.dma_start(cacc_out.ap(), cacc[:])
            nc.sync.dma_start(pacc_out.ap(), pacc[:])

    nc.compile()
    return nc


_CACHED_NC = None


def _get_program() -> bass.Bass:
    global _CACHED_NC
    if _CACHED_NC is None:
        _CACHED_NC = build_program()
    return _CACHED_NC


def shard_inputs(pred: np.ndarray, target: np.ndarray) -> list[dict]:
    in_maps = []
    for i in range(N_CORES):
        sl = slice(i * B_SHARD, (i + 1) * B_SHARD)
        in_maps.append(
            {
                "pred": np.ascontiguousarray(pred[sl]).reshape(ROWS, W),
                "target": np.ascontiguousarray(target[sl]).reshape(ROWS, W),
            }
        )
    return in_maps


def reduce_partials(results: list[dict]) -> np.ndarray:
    tot_sq = 0.0
    tot_abs = 0.0
    for r in results:
        tot_sq += r["cacc"].astype(np.float64).sum()
        tot_abs += r["pacc"].astype(np.float64).sum()
    loss = tot_sq / (B * H * CW) + tot_abs / (B * H * PW)
    return np.asarray(loss, dtype=np.float32)


def kernel(pred: np.ndarray, target: np.ndarray) -> np.ndarray:
    pred = np.asarray(pred, dtype=np.float32)
    target = np.asarray(target, dtype=np.float32)
    nc = _get_program()
    in_maps = shard_inputs(pred, target)
    res = run_bass_kernel_spmd(nc, in_maps, list(range(N_CORES)))
    return reduce_partials(res.results)


# revision 29
# speedup vs baseline: 1.2044x; 1.2044x over previous
"""Trainium2 Bass kernel for CombinedKSpaceRowwiseMSELoss.

loss = mean((pred-target)^2 over central cols) summed over both channels'
       means + mean(|pred-target| over periphery cols) likewise.

Strategy: pure data parallel over the batch dim — 32 batches / 8 cores =
4 batches per core. Each core streams its [5120, 640] f32 shard of pred and
target through SBUF in 5 big tiles (128 partitions x 8 rows x 640 cols),
computes diff on the vector engine, fused square+row-accumulate on the vector
engine (tensor_tensor_reduce) for the central columns, and fused
abs+row-accumulate on the scalar engine (activation Abs with accum_out) for
the periphery columns. Per-tile per-partition partial sums land in two small
accumulator tiles that are DMA'd out; the final ~3K-element reduction and
normalization happen on the host.
"""

import sys

for _p in ("/opt/trn_rl_repo",):
    if _p not in sys.path:
        sys.path.insert(0, _p)

import numpy as np
from contextlib import ExitStack

import concourse.bass as bass
import concourse.tile as tile
from concourse import bacc, mybir
from concourse.bass_utils import run_bass_kernel_spmd

N_CORES = 8
B, C, H, W = 32, 2, 640, 640
B_SHARD = B // N_CORES          # 4 batch elements per core
ROWS = B_SHARD * C * H          # 5120 rows per core
P = 128                         # SBUF partitions
RPP = 5                         # rows per partition per big tile
TILE_ROWS = P * RPP             # 1024
T = ROWS // TILE_ROWS           # 5 big tiles per core
CW = int(W * 0.25)              # 160 central cols
CS = (W - CW) // 2              # 240
CE = CS + CW                    # 400
PW = W - CW                     # 480 periphery cols per row

F32 = mybir.dt.float32


def build_program(
    loop_n: int = 1,
    mode: str = "full",
    rpp: int = RPP,
    io_bufs: int = 3,
    rings: str = "split",
) -> bass.Bass:
    T = ROWS // (P * rpp)
    nc = bacc.Bacc("TRN2", target_bir_lowering=False, debug=False)

    pred = nc.dram_tensor("pred", [ROWS, W], F32, kind="ExternalInput")
    tgt = nc.dram_tensor("target", [ROWS, W], F32, kind="ExternalInput")
    cacc_out = nc.dram_tensor("cacc", [P, T], F32, kind="ExternalOutput")
    pacc_out = nc.dram_tensor("pacc", [P, 2 * T], F32, kind="ExternalOutput")

    # Tile t, partition p holds rows t*P*rpp + p*rpp .. +rpp-1, contiguous in DRAM.
    pred_v = pred.ap().rearrange("(t p r) w -> t p (r w)", p=P, r=rpp)
    tgt_v = tgt.ap().rearrange("(t p r) w -> t p (r w)", p=P, r=rpp)

    with tile.TileContext(nc) as tc:
        with ExitStack() as ctx:
            io_pool = ctx.enter_context(tc.tile_pool(name="io", bufs=io_bufs))
            work_pool = ctx.enter_context(tc.tile_pool(name="work", bufs=3))
            acc_pool = ctx.enter_context(tc.tile_pool(name="acc", bufs=1))

            cacc = acc_pool.tile([P, T], F32)
            pacc = acc_pool.tile([P, 2 * T], F32)
            if mode != "full":
                nc.vector.memset(cacc[:], 0.0)
                nc.vector.memset(pacc[:], 0.0)
            fixed_io = None
            if mode not in ("full", "dma"):
                fpred = acc_pool.tile([P, rpp * W], F32, tag="fpred")
                ftgt = acc_pool.tile([P, rpp * W], F32, tag="ftgt")
                fixed_io = (fpred, ftgt)
                nc.vector.memset(fpred[:], 1.0)
                nc.vector.memset(ftgt[:], 2.0)

            def body():
                for t in range(T):
                    emit_tile(t)

            def emit_tile(t):
                # pred on the SP HWDGE ring, target on the ACT HWDGE ring —
                # two issuers so descriptor posting isn't serialized.
                if fixed_io is not None:
                    pt, gt = fixed_io
                else:
                    pt = io_pool.tile([P, rpp * W], F32, tag="pred")
                    gt = io_pool.tile([P, rpp * W], F32, tag="tgt")
                    if rings == "split":
                        nc.sync.dma_start(pt[:], pred_v[t])
                        nc.scalar.dma_start(gt[:], tgt_v[t])
                    elif rings == "sp":
                        nc.sync.dma_start(pt[:], pred_v[t])
                        nc.sync.dma_start(gt[:], tgt_v[t])
                    else:  # alternate rings by tile parity
                        e0, e1 = (nc.sync, nc.scalar) if t % 2 == 0 else (nc.scalar, nc.sync)
                        e0.dma_start(pt[:], pred_v[t])
                        e1.dma_start(gt[:], tgt_v[t])
                if mode == "dma":
                    return

                do_sub = mode in ("full", "compute", "sub", "subsq")
                do_sq = mode in ("full", "compute", "sq", "subsq")
                do_abs = mode in ("full", "compute", "absred")

                if do_sub:
                    diff = work_pool.tile([P, rpp * W], F32, tag="diff")
                    nc.vector.tensor_sub(diff[:], pt[:], gt[:])
                    d3 = diff[:].rearrange("p (r w) -> p r w", w=W)
                else:
                    d3 = gt[:].rearrange("p (r w) -> p r w", w=W)

                if do_sq:
                    # central: accum += sum over rows of (diff*diff) on ACT
                    sq = work_pool.tile([P, rpp * CW], F32, tag="sq")
                    nc.scalar.activation(
                        sq[:].rearrange("p (r w) -> p r w", w=CW),
                        d3[:, :, CS:CE],
                        mybir.ActivationFunctionType.Square,
                        accum_out=cacc[:, t : t + 1],
                    )

                if do_abs:
                    # periphery: abs-sum on DVE (fused |x| reduce, no scratch)
                    nc.vector.tensor_reduce(
                        pacc[:, 2 * t : 2 * t + 1],
                        d3[:, :, 0:CS],
                        axis=mybir.AxisListType.XY,
                        op=mybir.AluOpType.add,
                        apply_absolute_value=True,
                    )
                    nc.vector.tensor_reduce(
                        pacc[:, 2 * t + 1 : 2 * t + 2],
                        d3[:, :, CE:],
                        axis=mybir.AxisListType.XY,
                        op=mybir.AluOpType.add,
                        apply_absolute_value=True,
                    )

            if loop_n > 1:
                with tc.For_i(0, loop_n, 1):
                    body()
            else:
                body()

            nc.sync.dma_start(cacc_out.ap(), cacc[:])
            nc.sync.dma_start(pacc_out.ap(), pacc[:])

    nc.compile()
    return nc


_CACHED_NC = None


def _get_program() -> bass.Bass:
    global _CACHED_NC
    if _CACHED_NC is None:
        _CACHED_NC = build_program()
    return _CACHED_NC


def shard_inputs(pred: np.ndarray, target: np.ndarray) -> list[dict]:
    in_maps = []
    for i in range(N_CORES):
        sl = slice(i * B_SHARD, (i + 1) * B_SHARD)
        in_maps.append(
            {
                "pred": np.ascontiguousarray(pred[sl]).reshape(ROWS, W),
                "target": np.ascontiguousarray(target[sl]).reshape(ROWS, W),
            }
        )
    return in_maps


def reduce_partials(results: list[dict]) -> np.ndarray:
    tot_sq = 0.0
    tot_abs = 0.0
    for r in results:
        tot_sq += r["cacc"].astype(np.float64).sum()
        tot_abs += r["pacc"].astype(np.float64).sum()
    loss = tot_sq / (B * H * CW) + tot_abs / (B * H * PW)
    return np.asarray(loss, dtype=np.float32)


def kernel(pred: np.ndarray, target: np.ndarray) -> np.ndarray:
    pred = np.asarray(pred, dtype=np.float32)
    target = np.asarray(target, dtype=np.float32)
    nc = _get_program()
    in_maps = shard_inputs(pred, target)
    res = run_bass_kernel_spmd(nc, in_maps, list(range(N_CORES)))
    return reduce_partials(res.results)
